# revision 5
# baseline (speedup 1.0000x reference)
# Multi-head attention (B=4, L=2048, E=256, H=8) on 8 TRN2 NeuronCores.
#
# Sharding: core c handles batch b = c//2 and head group g = c%2 (heads
# 4g..4g+3).  Each core computes the partial output
#   sum_{h in group} softmax(x M_h x^T) (x N_h)
# for its batch, where the host pre-folds the per-head weights:
#   M_h = Wq_h Wk_h^T / sqrt(E)   (so scores = q k^T/16 = x M_h x^T)
#   N_h = Wv_h Wout_h             (so attn @ v @ Wout_h = (attn @ x) N_h)
# The host adds the two head-group partials per batch.  Folding removes
# the k and v projections from the device entirely.
#
# Per-core dataflow (big matmuls in float32r, full PE rate at N>=256):
#   xT = x^T (PE transpose);  x kept resident in natural layout too
#   uT_h = M_h^T x^T                ([256, 2048], e2 on partitions)
#   per 512-wide qi block, streaming over 16 kj tiles:
#     sT   = xT[:,kj]^T uT   (PSUM [128kj, 512qi])   == scores^T
#     pT   = exp(sT)         (ACT, PSUM->SBUF)
#     colacc += pT           (DVE running sum for the softmax denominator)
#     wT  += x[kj]^T pT      (PSUM [128e, 512qi] = (p @ x)^T, acc over kj)
#   rowsum = colacc^T @ ones (PE, [128qi, 1]) ; recip = 1/rowsum (DVE)
#   out[qi] += (wT^T @ N_h) * recip   (per-partition scale on ACT)
# Scores never touch HBM; softmax normalization is applied after the
# output projection (row scaling commutes with right-multiplication).
# SBUF tiles feeding fp32r matmuls are declared float32r (the BIR
# verifier requires producers to round to fp32r); DVE/ACT consumers
# read them bitcast back to fp32.

import numpy as np

B, L, E, H = 4, 2048, 256, 8
HL = H // 2          # heads per core
LT = L // 128        # 16 row tiles
QB = 512             # qi block width
NQB = L // QB        # 4
KT = L // 128        # 16 kj tiles

_cache = {}


def _build_nc():
    import concourse.mybir as mybir
    from concourse import bacc
    from concourse.tile import TileContext
    from concourse.masks import make_identity

    F32 = mybir.dt.float32
    F32R = mybir.dt.float32r
    Exp = mybir.ActivationFunctionType.Exp
    Copy = mybir.ActivationFunctionType.Copy

    def f(ap):  # read a float32r tile as plain f32 (same bits)
        return ap.bitcast(F32)

    nc = bacc.Bacc(None, target_bir_lowering=False)

    x_d = nc.dram_tensor("x", [L, E], F32, kind="ExternalInput")
    m_d = nc.dram_tensor("m", [E, HL * E], F32, kind="ExternalInput")
    n_d = nc.dram_tensor("n", [E, HL * E], F32, kind="ExternalInput")
    out_d = nc.dram_tensor("out", [L, E], F32, kind="ExternalOutput")

    with TileContext(nc) as tc:
        with (
            tc.tile_pool(name="const", bufs=1) as cpool,
            tc.tile_pool(name="head", bufs=2) as hpool,
            tc.tile_pool(name="work", bufs=2) as wpool,
            tc.tile_pool(name="ps_s", bufs=2, space="PSUM") as ps_s,
            tc.tile_pool(name="ps_ao", bufs=4, space="PSUM") as ps_ao,
            tc.tile_pool(name="ps_misc", bufs=2, space="PSUM") as ps_misc,
        ):
            ident = cpool.tile([128, 128], F32, name="ident")
            make_identity(nc, ident)
            ones = cpool.tile([128, 1], F32, name="ones")
            nc.gpsimd.memset(ones, 1.0)

            # ---- x load (natural, resident) + transpose to xT ----
            xn = [cpool.tile([128, E], F32R, name=f"xn{t}") for t in range(LT)]
            xT = [cpool.tile([128, L], F32R, name=f"xT{i}") for i in range(2)]
            for t in range(LT):
                nc.sync.dma_start(xn[t], x_d[t * 128:(t + 1) * 128, :].bitcast(F32R))
                for eh in range(2):
                    pst = ps_misc.tile([128, 128], F32, name="xtp", tag="misc")
                    nc.tensor.transpose(pst, f(xn[t][:, eh * 128:(eh + 1) * 128]),
                                        ident)
                    nc.vector.tensor_copy(xT[eh][:, t * 128:(t + 1) * 128], pst)

            # ---- folded weights (resident, float32r) ----
            m_sb = [cpool.tile([128, HL * E], F32R, name=f"m{i}") for i in range(2)]
            n_sb = [cpool.tile([128, HL * E], F32R, name=f"n{i}") for i in range(2)]
            for i in range(2):
                nc.sync.dma_start(m_sb[i], m_d[i * 128:(i + 1) * 128, :].bitcast(F32R))
                nc.sync.dma_start(n_sb[i], n_d[i * 128:(i + 1) * 128, :].bitcast(F32R))

            out_acc = [cpool.tile([128, E], F32, name=f"oacc{t}") for t in range(LT)]

            for h in range(HL):
                # ---- uT_h = M_h^T x^T  ([256, 2048] as 2 e2-halves) ----
                uT = [hpool.tile([128, L], F32R, name=f"uT{eh}", tag=f"uT{eh}")
                      for eh in range(2)]
                for eh in range(2):
                    for nb in range(NQB):
                        ps = ps_misc.tile([128, QB], F32, name="ups", tag="misc")
                        for ih in range(2):
                            nc.tensor.matmul(
                                ps,
                                m_sb[ih][:, h * E + eh * 128:h * E + (eh + 1) * 128],
                                xT[ih][:, nb * QB:(nb + 1) * QB],
                                start=(ih == 0), stop=(ih == 1),
                            )
                        nc.scalar.activation(uT[eh][:, nb * QB:(nb + 1) * QB], ps,
                                             Copy)

                # ---- attention, one 512-wide qi block at a time ----
                for qb in range(NQB):
                    colacc = wpool.tile([128, QB], F32, name="colacc", tag="colacc")
                    w_ps = [ps_ao.tile([128, QB], F32, name=f"wps{eh}", tag="ao")
                            for eh in range(2)]
                    for t in range(KT):
                        s_ps = ps_s.tile([128, QB], F32, name="sps", tag="s")
                        for eh in range(2):
                            nc.tensor.matmul(
                                s_ps,
                                xT[eh][:, t * 128:(t + 1) * 128],
                                uT[eh][:, qb * QB:(qb + 1) * QB],
                                start=(eh == 0), stop=(eh == 1),
                            )
                        pt = wpool.tile([128, QB], F32R, name="pt", tag="pt", bufs=3)
                        nc.scalar.activation(pt, s_ps, Exp)
                        if t == 0:
                            nc.vector.tensor_copy(colacc, f(pt))
                        else:
                            nc.vector.tensor_add(colacc, colacc, f(pt))
                        for eh in range(2):
                            nc.tensor.matmul(
                                w_ps[eh],
                                xn[t][:, eh * 128:(eh + 1) * 128],
                                pt,
                                start=(t == 0), stop=(t == KT - 1),
                            )
                    wT = [wpool.tile([128, QB], F32R, name=f"wT{eh}", tag=f"wT{eh}")
                          for eh in range(2)]
                    for eh in range(2):
                        nc.vector.tensor_copy(wT[eh], w_ps[eh])
                    for j in range(QB // 128):
                        rs_ps = ps_misc.tile([128, 1], F32, name="rsps", tag="misc")
                        nc.tensor.matmul(rs_ps, colacc[:, j * 128:(j + 1) * 128],
                                         ones, start=True, stop=True)
                        recip = wpool.tile([128, 1], F32, name="recip", tag="recip",
                                           bufs=4)
                        nc.vector.reciprocal(recip, rs_ps)
                        pj_ps = ps_misc.tile([128, E], F32, name="pjps", tag="misc")
                        for eh in range(2):
                            nc.tensor.matmul(
                                pj_ps,
                                wT[eh][:, j * 128:(j + 1) * 128],
                                n_sb[eh][:, h * E:(h + 1) * E],
                                start=(eh == 0), stop=(eh == 1),
                            )
                        gt = qb * (QB // 128) + j
                        if h == 0:
                            nc.scalar.activation(out_acc[gt], pj_ps, Copy, scale=recip)
                        else:
                            tmp = wpool.tile([128, E], F32, name="ptmp", tag="ptmp")
                            nc.scalar.activation(tmp, pj_ps, Copy, scale=recip)
                            nc.vector.tensor_add(out_acc[gt], out_acc[gt], tmp)

            for t in range(LT):
                nc.sync.dma_start(out_d[t * 128:(t + 1) * 128, :], out_acc[t])

    nc.compile()
    return nc


def _get_nc():
    if "nc" not in _cache:
        _cache["nc"] = _build_nc()
    return _cache["nc"]


def _in_maps(x, W_qkv, W_out):
    x = np.ascontiguousarray(np.asarray(x, dtype=np.float32))
    W_qkv = np.asarray(W_qkv, dtype=np.float32)
    W_out = np.asarray(W_out, dtype=np.float32)

    # Host-side weight folding (float64 for exactness, cast to f32):
    #   M_h = Wq_h Wk_h^T / sqrt(E),   N_h = Wv_h Wout_h
    Wq = W_qkv[:, 0:H * E].astype(np.float64)
    Wk = W_qkv[:, H * E:2 * H * E].astype(np.float64)
    Wv = W_qkv[:, 2 * H * E:3 * H * E].astype(np.float64)
    Wo = W_out.astype(np.float64)
    scale = 1.0 / np.sqrt(E)
    M = np.empty((H, E, E), np.float64)
    N = np.empty((H, E, E), np.float64)
    for h in range(H):
        M[h] = (Wq[:, h * E:(h + 1) * E] @ Wk[:, h * E:(h + 1) * E].T) * scale
        N[h] = Wv[:, h * E:(h + 1) * E] @ Wo[h * E:(h + 1) * E, :]

    maps = []
    for c in range(2 * B):
        b, g = c // 2, c % 2
        hs = HL * g  # first head of this core's group
        mcat = np.concatenate([M[hs + i] for i in range(HL)], axis=1)
        ncat = np.concatenate([N[hs + i] for i in range(HL)], axis=1)
        maps.append({
            "x": np.ascontiguousarray(x[b]),
            "m": np.ascontiguousarray(mcat.astype(np.float32)),
            "n": np.ascontiguousarray(ncat.astype(np.float32)),
        })
    return maps


def kernel(x, W_qkv, W_out, _trace=False):
    from concourse.bass_utils import run_bass_kernel_spmd

    nc = _get_nc()
    maps = _in_maps(x, W_qkv, W_out)
    res = run_bass_kernel_spmd(nc, maps, core_ids=list(range(2 * B)),
                               trace=_trace)
    _cache["last_result"] = res
    outs = [m["out"] for m in res.results]
    full = np.stack([outs[2 * b] + outs[2 * b + 1] for b in range(B)])
    return full.astype(np.float32)


# revision 8
# speedup vs baseline: 1.1313x; 1.1313x over previous
# Multi-head attention (B=4, L=2048, E=256, H=8) on 8 TRN2 NeuronCores.
#
# Sharding: core c handles batch b = c//2 and head group g = c%2 (heads
# 4g..4g+3).  Each core computes the partial output
#   sum_{h in group} softmax(x M_h x^T) (x N_h)
# for its batch, where the host pre-folds the per-head weights:
#   M_h = Wq_h Wk_h^T / sqrt(E)   (so scores = q k^T/16 = x M_h x^T)
#   N_h = Wv_h Wout_h             (so attn @ v @ Wout_h = (attn @ x) N_h)
# The host adds the two head-group partials per batch.  Folding removes
# the k and v projections from the device entirely.
#
# Per-core dataflow (big matmuls in float32r, full PE rate at N>=256):
#   xT = x^T (PE transpose);  x kept resident in natural layout too
#   uT_h = M_h^T x^T                ([256, 2048], e2 on partitions)
#   per 512-wide qi block, streaming over 16 kj tiles:
#     sT   = xT[:,kj]^T uT   (PSUM [128kj, 512qi])   == scores^T
#     pT   = exp(sT)         (ACT, PSUM->SBUF)
#     colacc += pT           (DVE running sum for the softmax denominator)
#     wT  += x[kj]^T pT      (PSUM [128e, 512qi] = (p @ x)^T, acc over kj)
#   rowsum = colacc^T @ ones (PE, [128qi, 1]) ; recip = 1/rowsum (DVE)
#   out[qi] += (wT^T @ N_h) * recip   (per-partition scale on ACT)
# Scores never touch HBM; softmax normalization is applied after the
# output projection (row scaling commutes with right-multiplication).
# SBUF tiles feeding fp32r matmuls are declared float32r (the BIR
# verifier requires producers to round to fp32r); DVE/ACT consumers
# read them bitcast back to fp32.

import numpy as np

B, L, E, H = 4, 2048, 256, 8
HL = H // 2          # heads per core
LT = L // 128        # 16 row tiles
QB = 512             # qi block width
NQB = L // QB        # 4
KT = L // 128        # 16 kj tiles

_cache = {}


def _build_nc():
    import concourse.mybir as mybir
    from concourse import bacc
    from concourse.tile import TileContext
    from concourse.masks import make_identity

    F32 = mybir.dt.float32
    F32R = mybir.dt.float32r
    Exp = mybir.ActivationFunctionType.Exp
    Copy = mybir.ActivationFunctionType.Copy

    def f(ap):  # read a float32r tile as plain f32 (same bits)
        return ap.bitcast(F32)

    nc = bacc.Bacc(None, target_bir_lowering=False)

    x_d = nc.dram_tensor("x", [L, E], F32, kind="ExternalInput")
    m_d = nc.dram_tensor("m", [E, HL * E], F32, kind="ExternalInput")
    n_d = nc.dram_tensor("n", [E, HL * E], F32, kind="ExternalInput")
    out_d = nc.dram_tensor("out", [L, E], F32, kind="ExternalOutput")

    with TileContext(nc) as tc:
        with (
            tc.tile_pool(name="const", bufs=1) as cpool,
            tc.tile_pool(name="head", bufs=2) as hpool,
            tc.tile_pool(name="work", bufs=2) as wpool,
            tc.tile_pool(name="ps_s", bufs=3, space="PSUM") as ps_s,
            tc.tile_pool(name="ps_ao", bufs=3, space="PSUM") as ps_ao,
            tc.tile_pool(name="ps_misc", bufs=2, space="PSUM") as ps_misc,
        ):
            ident = cpool.tile([128, 128], F32, name="ident")
            make_identity(nc, ident)
            ones = cpool.tile([128, 1], F32, name="ones")
            nc.gpsimd.memset(ones, 1.0)

            # ---- x load (natural, resident) + transpose to xT ----
            xn = [cpool.tile([128, E], F32R, name=f"xn{t}") for t in range(LT)]
            xT = [cpool.tile([128, L], F32R, name=f"xT{i}") for i in range(2)]
            for t in range(LT):
                nc.sync.dma_start(xn[t], x_d[t * 128:(t + 1) * 128, :].bitcast(F32R))
                for eh in range(2):
                    pst = ps_misc.tile([128, 128], F32, name="xtp", tag="misc")
                    nc.tensor.transpose(pst, f(xn[t][:, eh * 128:(eh + 1) * 128]),
                                        ident)
                    nc.vector.tensor_copy(xT[eh][:, t * 128:(t + 1) * 128], pst)

            # ---- folded weights (resident, float32r) ----
            m_sb = [cpool.tile([128, HL * E], F32R, name=f"m{i}") for i in range(2)]
            n_sb = [cpool.tile([128, HL * E], F32R, name=f"n{i}") for i in range(2)]
            for i in range(2):
                nc.sync.dma_start(m_sb[i], m_d[i * 128:(i + 1) * 128, :].bitcast(F32R))
                nc.sync.dma_start(n_sb[i], n_d[i * 128:(i + 1) * 128, :].bitcast(F32R))

            out_acc = [cpool.tile([128, E], F32, name=f"oacc{t}") for t in range(LT)]

            for h in range(HL):
                # ---- uT_h = M_h^T x^T  ([256, 2048] as 2 e2-halves) ----
                uT = [hpool.tile([128, L], F32R, name=f"uT{eh}", tag=f"uT{eh}")
                      for eh in range(2)]
                for eh in range(2):
                    for nb in range(NQB):
                        ps = ps_misc.tile([128, QB], F32, name="ups", tag="misc")
                        for ih in range(2):
                            nc.tensor.matmul(
                                ps,
                                m_sb[ih][:, h * E + eh * 128:h * E + (eh + 1) * 128],
                                xT[ih][:, nb * QB:(nb + 1) * QB],
                                start=(ih == 0), stop=(ih == 1),
                            )
                        nc.scalar.activation(uT[eh][:, nb * QB:(nb + 1) * QB], ps,
                                             Copy)

                # ---- attention, one 512-wide qi block at a time ----
                for qb in range(NQB):
                    colacc = wpool.tile([128, QB], F32, name="colacc", tag="colacc")
                    w_ps = [ps_ao.tile([128, QB], F32, name=f"wps{eh}", tag="ao")
                            for eh in range(2)]
                    for t in range(KT):
                        s_ps = ps_s.tile([128, QB], F32, name="sps", tag="s")
                        for eh in range(2):
                            nc.tensor.matmul(
                                s_ps,
                                xT[eh][:, t * 128:(t + 1) * 128],
                                uT[eh][:, qb * QB:(qb + 1) * QB],
                                start=(eh == 0), stop=(eh == 1),
                            )
                        pt = wpool.tile([128, QB], F32R, name="pt", tag="pt", bufs=4)
                        nc.scalar.activation(pt, s_ps, Exp)
                        if t == 0:
                            nc.vector.tensor_copy(colacc, f(pt))
                        else:
                            nc.vector.tensor_add(colacc, colacc, f(pt))
                        for eh in range(2):
                            nc.tensor.matmul(
                                w_ps[eh],
                                xn[t][:, eh * 128:(eh + 1) * 128],
                                pt,
                                start=(t == 0), stop=(t == KT - 1),
                            )
                    wT = [wpool.tile([128, QB], F32R, name=f"wT{eh}", tag=f"wT{eh}")
                          for eh in range(2)]
                    for eh in range(2):
                        nc.vector.tensor_copy(wT[eh], w_ps[eh])
                    for j in range(QB // 128):
                        rs_ps = ps_misc.tile([128, 1], F32, name="rsps", tag="misc")
                        nc.tensor.matmul(rs_ps, colacc[:, j * 128:(j + 1) * 128],
                                         ones, start=True, stop=True)
                        recip = wpool.tile([128, 1], F32, name="recip", tag="recip",
                                           bufs=4)
                        nc.vector.reciprocal(recip, rs_ps)
                        pj_ps = ps_misc.tile([128, E], F32, name="pjps", tag="misc")
                        for eh in range(2):
                            nc.tensor.matmul(
                                pj_ps,
                                wT[eh][:, j * 128:(j + 1) * 128],
                                n_sb[eh][:, h * E:(h + 1) * E],
                                start=(eh == 0), stop=(eh == 1),
                            )
                        gt = qb * (QB // 128) + j
                        if h == 0:
                            nc.vector.tensor_scalar_mul(out_acc[gt], pj_ps, recip)
                        else:
                            nc.vector.scalar_tensor_tensor(
                                out_acc[gt], pj_ps, recip, out_acc[gt],
                                op0=mybir.AluOpType.mult, op1=mybir.AluOpType.add)

            for t in range(LT):
                nc.sync.dma_start(out_d[t * 128:(t + 1) * 128, :], out_acc[t])

    nc.compile()
    return nc


def _get_nc():
    if "nc" not in _cache:
        _cache["nc"] = _build_nc()
    return _cache["nc"]


def _in_maps(x, W_qkv, W_out):
    x = np.ascontiguousarray(np.asarray(x, dtype=np.float32))
    W_qkv = np.asarray(W_qkv, dtype=np.float32)
    W_out = np.asarray(W_out, dtype=np.float32)

    # Host-side weight folding (float64 for exactness, cast to f32):
    #   M_h = Wq_h Wk_h^T / sqrt(E),   N_h = Wv_h Wout_h
    Wq = W_qkv[:, 0:H * E].astype(np.float64)
    Wk = W_qkv[:, H * E:2 * H * E].astype(np.float64)
    Wv = W_qkv[:, 2 * H * E:3 * H * E].astype(np.float64)
    Wo = W_out.astype(np.float64)
    scale = 1.0 / np.sqrt(E)
    M = np.empty((H, E, E), np.float64)
    N = np.empty((H, E, E), np.float64)
    for h in range(H):
        M[h] = (Wq[:, h * E:(h + 1) * E] @ Wk[:, h * E:(h + 1) * E].T) * scale
        N[h] = Wv[:, h * E:(h + 1) * E] @ Wo[h * E:(h + 1) * E, :]

    maps = []
    for c in range(2 * B):
        b, g = c // 2, c % 2
        hs = HL * g  # first head of this core's group
        mcat = np.concatenate([M[hs + i] for i in range(HL)], axis=1)
        ncat = np.concatenate([N[hs + i] for i in range(HL)], axis=1)
        maps.append({
            "x": np.ascontiguousarray(x[b]),
            "m": np.ascontiguousarray(mcat.astype(np.float32)),
            "n": np.ascontiguousarray(ncat.astype(np.float32)),
        })
    return maps


def kernel(x, W_qkv, W_out, _trace=False):
    from concourse.bass_utils import run_bass_kernel_spmd

    nc = _get_nc()
    maps = _in_maps(x, W_qkv, W_out)
    res = run_bass_kernel_spmd(nc, maps, core_ids=list(range(2 * B)),
                               trace=_trace)
    _cache["last_result"] = res
    outs = [m["out"] for m in res.results]
    full = np.stack([outs[2 * b] + outs[2 * b + 1] for b in range(B)])
    return full.astype(np.float32)


# revision 9
# speedup vs baseline: 1.1444x; 1.0116x over previous
# Multi-head attention (B=4, L=2048, E=256, H=8) on 8 TRN2 NeuronCores.
#
# Sharding: core c handles batch b = c//2 and head group g = c%2 (heads
# 4g..4g+3).  Each core computes the partial output
#   sum_{h in group} softmax(x M_h x^T) (x N_h)
# for its batch, where the host pre-folds the per-head weights:
#   M_h = Wq_h Wk_h^T / sqrt(E)   (so scores = q k^T/16 = x M_h x^T)
#   N_h = Wv_h Wout_h             (so attn @ v @ Wout_h = (attn @ x) N_h)
# The host adds the two head-group partials per batch.  Folding removes
# the k and v projections from the device entirely.  The host also
# supplies x^T so the device does no transposes.
#
# Per-core dataflow (big matmuls in float32r, full PE rate at N>=256):
#   uT_h = M_h^T x^T                ([256, 2048], e2 on partitions)
#   per 512-wide qi block, streaming over 16 kj tiles:
#     sT   = xT[:,kj]^T uT   (PSUM [128kj, 512qi])   == scores^T
#     pT   = exp(sT)         (ACT, PSUM->SBUF)
#     colacc += pT           (DVE running sum for the softmax denominator)
#     wT  += x[kj]^T pT      (PSUM [128e, 512qi] = (p @ x)^T, acc over kj)
#   rowsum = colacc^T @ ones (PE, [128qi, 1]) ; recip = 1/rowsum (DVE)
#   out[qi] += (wT^T @ N_h) * recip   (fused scale+add on DVE)
# Scores never touch HBM; softmax normalization is applied after the
# output projection (row scaling commutes with right-multiplication).
# SBUF tiles feeding fp32r matmuls are declared float32r (the BIR
# verifier requires producers to round to fp32r); DVE/ACT consumers
# read them bitcast back to fp32.

import numpy as np

B, L, E, H = 4, 2048, 256, 8
HL = H // 2          # heads per core
LT = L // 128        # 16 row tiles
QB = 512             # qi block width
NQB = L // QB        # 4
KT = L // 128        # 16 kj tiles

_cache = {}


def _build_nc():
    import concourse.mybir as mybir
    from concourse import bacc
    from concourse.tile import TileContext

    F32 = mybir.dt.float32
    F32R = mybir.dt.float32r
    Exp = mybir.ActivationFunctionType.Exp

    def f(ap):  # read a float32r tile as plain f32 (same bits)
        return ap.bitcast(F32)

    nc = bacc.Bacc(None, target_bir_lowering=False)

    x_d = nc.dram_tensor("x", [L, E], F32, kind="ExternalInput")
    xt_d = nc.dram_tensor("xt", [E, L], F32, kind="ExternalInput")
    m_d = nc.dram_tensor("m", [E, HL * E], F32, kind="ExternalInput")
    n_d = nc.dram_tensor("n", [E, HL * E], F32, kind="ExternalInput")
    out_d = nc.dram_tensor("out", [L, E], F32, kind="ExternalOutput")

    with TileContext(nc) as tc:
        with (
            tc.tile_pool(name="const", bufs=1) as cpool,
            tc.tile_pool(name="head", bufs=2) as hpool,
            tc.tile_pool(name="work", bufs=2) as wpool,
            tc.tile_pool(name="ps_s", bufs=3, space="PSUM") as ps_s,
            tc.tile_pool(name="ps_ao", bufs=3, space="PSUM") as ps_ao,
            tc.tile_pool(name="ps_misc", bufs=2, space="PSUM") as ps_misc,
        ):
            ones = cpool.tile([128, 1], F32, name="ones")
            nc.vector.memset(ones, 1.0)

            # ---- x (natural + transposed, resident, float32r) ----
            xT = [cpool.tile([128, L], F32R, name=f"xT{i}") for i in range(2)]
            for i in range(2):
                for nb in range(NQB):
                    nc.sync.dma_start(
                        xT[i][:, nb * QB:(nb + 1) * QB],
                        xt_d[i * 128:(i + 1) * 128,
                             nb * QB:(nb + 1) * QB].bitcast(F32R))
            m_sb = [cpool.tile([128, HL * E], F32R, name=f"m{i}") for i in range(2)]
            for i in range(2):
                nc.sync.dma_start(m_sb[i], m_d[i * 128:(i + 1) * 128, :].bitcast(F32R))
            xn = [cpool.tile([128, E], F32R, name=f"xn{t}") for t in range(LT)]
            for t in range(LT):
                nc.sync.dma_start(xn[t], x_d[t * 128:(t + 1) * 128, :].bitcast(F32R))
            n_sb = [cpool.tile([128, HL * E], F32R, name=f"n{i}") for i in range(2)]
            for i in range(2):
                nc.sync.dma_start(n_sb[i], n_d[i * 128:(i + 1) * 128, :].bitcast(F32R))

            out_acc = [cpool.tile([128, E], F32, name=f"oacc{t}") for t in range(LT)]

            for h in range(HL):
                # ---- uT_h = M_h^T x^T  ([256, 2048] as 2 e2-halves) ----
                uT = [hpool.tile([128, L], F32R, name=f"uT{eh}", tag=f"uT{eh}")
                      for eh in range(2)]
                for eh in range(2):
                    for nb in range(NQB):
                        ps = ps_s.tile([128, QB], F32, name="ups", tag="s")
                        for ih in range(2):
                            nc.tensor.matmul(
                                ps,
                                m_sb[ih][:, h * E + eh * 128:h * E + (eh + 1) * 128],
                                xT[ih][:, nb * QB:(nb + 1) * QB],
                                start=(ih == 0), stop=(ih == 1),
                            )
                        nc.vector.tensor_copy(uT[eh][:, nb * QB:(nb + 1) * QB], ps)

                # ---- attention, one 512-wide qi block at a time ----
                for qb in range(NQB):
                    colacc = wpool.tile([128, QB], F32, name="colacc", tag="colacc")
                    w_ps = [ps_ao.tile([128, QB], F32, name=f"wps{eh}", tag="ao")
                            for eh in range(2)]
                    for t in range(KT):
                        s_ps = ps_s.tile([128, QB], F32, name="sps", tag="s")
                        for eh in range(2):
                            nc.tensor.matmul(
                                s_ps,
                                xT[eh][:, t * 128:(t + 1) * 128],
                                uT[eh][:, qb * QB:(qb + 1) * QB],
                                start=(eh == 0), stop=(eh == 1),
                            )
                        pt = wpool.tile([128, QB], F32R, name="pt", tag="pt", bufs=4)
                        nc.scalar.activation(pt, s_ps, Exp)
                        if t == 0:
                            nc.vector.tensor_copy(colacc, f(pt))
                        else:
                            nc.vector.tensor_add(colacc, colacc, f(pt))
                        for eh in range(2):
                            nc.tensor.matmul(
                                w_ps[eh],
                                xn[t][:, eh * 128:(eh + 1) * 128],
                                pt,
                                start=(t == 0), stop=(t == KT - 1),
                            )
                    wT = [wpool.tile([128, QB], F32R, name=f"wT{eh}", tag=f"wT{eh}")
                          for eh in range(2)]
                    for eh in range(2):
                        nc.vector.tensor_copy(wT[eh], w_ps[eh])
                    for j in range(QB // 128):
                        rs_ps = ps_s.tile([128, 1], F32, name="rsps", tag="s")
                        nc.tensor.matmul(rs_ps, colacc[:, j * 128:(j + 1) * 128],
                                         ones, start=True, stop=True)
                        recip = wpool.tile([128, 1], F32, name="recip", tag="recip",
                                           bufs=4)
                        nc.vector.reciprocal(recip, rs_ps)
                        pj_ps = ps_misc.tile([128, E], F32, name="pjps", tag="misc")
                        for eh in range(2):
                            nc.tensor.matmul(
                                pj_ps,
                                wT[eh][:, j * 128:(j + 1) * 128],
                                n_sb[eh][:, h * E:(h + 1) * E],
                                start=(eh == 0), stop=(eh == 1),
                            )
                        gt = qb * (QB // 128) + j
                        if h == 0:
                            nc.vector.tensor_scalar_mul(out_acc[gt], pj_ps, recip)
                        else:
                            nc.vector.scalar_tensor_tensor(
                                out_acc[gt], pj_ps, recip, out_acc[gt],
                                op0=mybir.AluOpType.mult, op1=mybir.AluOpType.add)

            for t in range(LT):
                nc.sync.dma_start(out_d[t * 128:(t + 1) * 128, :], out_acc[t])

    nc.compile()
    return nc


def _get_nc():
    if "nc" not in _cache:
        _cache["nc"] = _build_nc()
    return _cache["nc"]


def _in_maps(x, W_qkv, W_out):
    x = np.ascontiguousarray(np.asarray(x, dtype=np.float32))
    W_qkv = np.asarray(W_qkv, dtype=np.float32)
    W_out = np.asarray(W_out, dtype=np.float32)

    # Host-side weight folding (float64 for exactness, cast to f32):
    #   M_h = Wq_h Wk_h^T / sqrt(E),   N_h = Wv_h Wout_h
    Wq = W_qkv[:, 0:H * E].astype(np.float64)
    Wk = W_qkv[:, H * E:2 * H * E].astype(np.float64)
    Wv = W_qkv[:, 2 * H * E:3 * H * E].astype(np.float64)
    Wo = W_out.astype(np.float64)
    scale = 1.0 / np.sqrt(E)
    M = np.empty((H, E, E), np.float64)
    N = np.empty((H, E, E), np.float64)
    for h in range(H):
        M[h] = (Wq[:, h * E:(h + 1) * E] @ Wk[:, h * E:(h + 1) * E].T) * scale
        N[h] = Wv[:, h * E:(h + 1) * E] @ Wo[h * E:(h + 1) * E, :]

    maps = []
    for c in range(2 * B):
        b, g = c // 2, c % 2
        hs = HL * g  # first head of this core's group
        mcat = np.concatenate([M[hs + i] for i in range(HL)], axis=1)
        ncat = np.concatenate([N[hs + i] for i in range(HL)], axis=1)
        maps.append({
            "x": np.ascontiguousarray(x[b]),
            "xt": np.ascontiguousarray(x[b].T),
            "m": np.ascontiguousarray(mcat.astype(np.float32)),
            "n": np.ascontiguousarray(ncat.astype(np.float32)),
        })
    return maps


def kernel(x, W_qkv, W_out, _trace=False):
    from concourse.bass_utils import run_bass_kernel_spmd

    nc = _get_nc()
    maps = _in_maps(x, W_qkv, W_out)
    res = run_bass_kernel_spmd(nc, maps, core_ids=list(range(2 * B)),
                               trace=_trace)
    _cache["last_result"] = res
    outs = [m["out"] for m in res.results]
    full = np.stack([outs[2 * b] + outs[2 * b + 1] for b in range(B)])
    return full.astype(np.float32)
